# revision 44
# baseline (speedup 1.0000x reference)
"""Trainium2 Bass kernel for CapsuleFC EM-routing forward pass.

Shapes: x[256,64,128], current_act[256,64], W[64,128,32,128], num_iter=3.
Outputs: ncv[256,32,128], q[256,64,32], route_class_emb[256,64,32,128].

Strategy: data-parallel over batch across 8 cores (32 b per core), W replicated.
Per core, votes = einsum('bna,namd->bnmd') are computed once by streaming W
(bf16) through the PE and kept entirely in SBUF (bf16, 16.75 MB).  The three
EM-routing iterations then run out of SBUF: logits on DVE (mul + segmented
reduce), softmax on ACT (Exp with fused sum), weighted votes via per-m
tensor_scalar, and the sum over n on the PE (matmul against a 0/1 selector).
route_class_emb is the final weighted votes, cast-DMA'd out as fp32.

Partition layout for votes: partition p = (n mod 4)*32 + b, SBUF tile
g = n // 4 (16 tiles), free dim = (m, d) = 4096.
"""

import math
import numpy as np
import ml_dtypes
from contextlib import ExitStack

import concourse.bass as bass
import concourse.bacc as bacc
import concourse.mybir as mybir
from concourse.tile import TileContext
from concourse.bass_utils import run_bass_kernel_spmd

BF16 = ml_dtypes.bfloat16

B, NIN, DIN, M, DOUT = 256, 64, 128, 32, 128
NCORES = 8
BSH = B // NCORES          # 32 batch elements per core
MD = M * DOUT              # 4096
NT = NIN // 4              # 16 votes tiles (4 n's per tile)
SCALE = 1.0 / math.sqrt(DOUT)

F32 = mybir.dt.float32
BF = mybir.dt.bfloat16

# Tuning knobs.
# Of the 16 (b,n)-tiles per iteration: how many logits-reduces run on the
# Scalar engine (per-m Copy+accum) and how many logits-multiplies run on
# GpSimd, to offload the bottleneck Vector engine.
ACT_REDUCE_TILES = 0   # HW: ACT per-m accum reduce costs ~1.3us/m -> never
GPSIMD_MUL_TILES = 9   # logits multiplies offloaded to GpSimd
GPSIMD_RW_TILES = 6    # weighted-vote multiplies offloaded to GpSimd


def _bcast_m(ap, m):
    """View a [128, 1] AP as [128, m] broadcast along free (step 0)."""
    return bass.AP(ap.tensor, ap.offset, [ap.ap[0], [0, m]])


def _bcast_md(ap, m, d):
    """View a [128, m] AP as [128, m, d] with d broadcast (step 0)."""
    return bass.AP(ap.tensor, ap.offset, [ap.ap[0], [ap.ap[-1][0], m], [0, d]])


def build_nc(num_iter: int = 3) -> bass.Bass:
    nc = bacc.Bacc()

    xT = nc.dram_tensor("xT", [DIN, NIN * BSH], BF, kind="ExternalInput")
    actt = nc.dram_tensor("act_t", [128, NT], F32, kind="ExternalInput")
    Wt = nc.dram_tensor("Wt", [NIN, DIN, MD], BF, kind="ExternalInput")
    # sel4x4[p, j] = (p%32 == j%32): matmuls with this as lhsT produce the
    # over-n reduction replicated to all four 32-partition groups at once
    sel4x4 = nc.dram_tensor("sel4x4", [128, 128], BF, kind="ExternalInput")

    ncv_o = nc.dram_tensor("ncv_o", [BSH, MD], BF, kind="ExternalOutput")
    q_o = nc.dram_tensor("q_o", [NT, 128, M], F32, kind="ExternalOutput")
    rce_o = nc.dram_tensor("rce_o", [NT, 128, MD], BF, kind="ExternalOutput")

    W_anm = Wt.rearrange("n a k -> a n k")  # [128, 64, 4096] strided DRAM view

    with TileContext(nc) as tc, ExitStack() as ctx:
        pp = ctx.enter_context(tc.tile_pool(name="persist", bufs=1))
        V = pp.tile([128, NT * MD], BF)            # votes, 128 KB/partition
        xsb = pp.tile([128, NIN * BSH], BF)        # x^T, free = (n, b)
        asb = pp.tile([128, NT], F32)              # act per (partition, tile)
        s4x = pp.tile([128, 128], BF)              # replicating selector
        lg = pp.tile([128, NT * M], F32)           # logits, free = (g, m)

        nc.sync.dma_start(out=xsb[:, :], in_=xT[:, :])
        nc.sync.dma_start(out=asb[:, :], in_=actt[:, :])
        nc.sync.dma_start(out=s4x[:, :], in_=sel4x4[:, :])

        # ---------------- Phase 1: votes = x @ W, streamed over n ----------
        with tc.tile_pool(name="wstream", bufs=3) as wp, \
             tc.tile_pool(name="p1ps", bufs=2, space="PSUM") as psp:
            for g in range(NT):
                wtiles = []
                for half in range(2):
                    n0 = 4 * g + 2 * half
                    wtile = wp.tile([128, 2 * MD], BF, tag="w", name=f"w_{g}_{half}")
                    for nl in range(2):
                        nc.sync.dma_start(
                            out=wtile[:, nl * MD:(nl + 1) * MD],
                            in_=W_anm[:, n0 + nl, :])
                    wtiles.append(wtile)
                # psA covers (m,d) chunks 0-3, psB chunks 4-7; each is written
                # by all four n's (col groups) of this quad.
                psA = psp.tile([128, 4 * 512], F32, tag="ps", name=f"psA_{g}")
                psB = psp.tile([128, 4 * 512], F32, tag="ps", name=f"psB_{g}")
                for half in range(2):
                    wtile = wtiles[half]
                    for nl in range(2):
                        n = 4 * g + 2 * half + nl
                        j = 2 * half + nl  # col group -> partitions 32j..32j+31
                        lhsT = xsb[:, n * BSH:(n + 1) * BSH]
                        for c in range(8):
                            ps = psA if c < 4 else psB
                            cl = c % 4
                            nc.tensor.matmul(
                                ps[32 * j:32 * j + 32, cl * 512:(cl + 1) * 512],
                                lhsT=lhsT,
                                rhs=wtile[:, nl * MD + c * 512: nl * MD + (c + 1) * 512],
                                start=True, stop=True,
                                tile_position=(0, 32 * j),
                            )
                # both PSUM->SBUF copies on ScalarE: phase 1 is DMA-bound and
                # this keeps the Vector engine free for the iteration phase
                nc.scalar.copy(
                    V[:, g * MD: g * MD + 2048], psA[:, :])
                nc.scalar.copy(
                    V[:, g * MD + 2048: g * MD + 4096], psB[:, :])

        with tc.tile_pool(name="it", bufs=1) as ip, \
             tc.tile_pool(name="scr", bufs=3) as sp, \
             tc.tile_pool(name="sm", bufs=4) as smp, \
             tc.tile_pool(name="itps", bufs=1, space="PSUM") as ipp:

            # ---------------- ncv_0 = (sum_n V) / M -------------------------
            # matmul against the replicating selector: output [128, MD] holds
            # the per-b sum over n replicated across all 4 partition groups
            ncv_ps = ipp.tile([128, MD], F32, tag="ncvps")
            for g in range(NT):
                for c in range(8):
                    nc.tensor.matmul(
                        ncv_ps[:, c * 512:(c + 1) * 512],
                        lhsT=s4x[:, :],
                        rhs=V[:, g * MD + c * 512: g * MD + (c + 1) * 512],
                        start=(g == 0), stop=(g == NT - 1),
                    )

            def broadcast_ncv(ncv_ps, scale):
                # single PSUM->SBUF copy; `scale` folds the logits SCALE (and
                # the 1/M of the init step) so logits need no extra scaling
                ncvb = ip.tile([128, MD], BF, tag="ncvb", bufs=2)
                nc.scalar.mul(ncvb[:, :], ncv_ps[:, :], scale)
                return ncvb

            ncvb = broadcast_ncv(ncv_ps, SCALE / M)

            # ---------------- routing iterations ---------------------------
            for it in range(num_iter):
                last = (it == num_iter - 1)
                ncv_ps = ipp.tile([128, MD], F32, tag="ncvps")
                # spread offloaded tiles through the loop so the pipeline
                # keeps flowing (gpsimd muls on odd tiles, gpsimd rw on
                # even tiles, ACT reduces on multiples of 4)
                gps_tiles = set(list(range(1, NT, 2))[:GPSIMD_MUL_TILES])
                gps_rw_tiles = set(list(range(0, NT, 2))[:GPSIMD_RW_TILES])
                act_tiles = set(list(range(0, NT, 4))[:ACT_REDUCE_TILES])
                for g in range(NT):
                    Vg = V[:, g * MD:(g + 1) * MD]
                    lgg = lg[:, g * M:(g + 1) * M]
                    # logits: mul + segmented reduce over d.  Free layout of
                    # votes is (d, m), so the reduce uses a strided [p, m, d]
                    # view and the w-broadcast below has innermost step 1.
                    tmp = sp.tile([128, MD], BF, tag="scr")
                    mul_eng = nc.gpsimd if g in gps_tiles else nc.vector
                    mul_eng.tensor_mul(tmp[:, :], Vg, ncvb[:, :])
                    tmp3 = tmp.rearrange("p (m d) -> p m d", d=DOUT)
                    if g in act_tiles:
                        # per-m Copy with fused accumulator on ScalarE
                        for m in range(M):
                            nc.scalar.activation(
                                tmp3[:, m, :], tmp3[:, m, :],
                                mybir.ActivationFunctionType.Copy,
                                accum_out=lgg[:, m:m + 1])
                    else:
                        nc.vector.reduce_sum(lgg, tmp3,
                                             axis=mybir.AxisListType.X)
                    # softmax over m (free dim); logits are pre-scaled by
                    # SCALE (folded into ncvb), max comes out negated.
                    nmx = smp.tile([128, 1], F32, tag="nmx")
                    nc.vector.tensor_reduce(
                        nmx[:, :], lgg, axis=mybir.AxisListType.X,
                        op=mybir.AluOpType.max, negate=True)
                    eqf = smp.tile([128, M], F32, tag="eqf")
                    se = smp.tile([128, 1], F32, tag="se")
                    nc.scalar.activation(
                        eqf[:, :], lgg, mybir.ActivationFunctionType.Exp,
                        bias=nmx[:, 0:1], scale=1.0, accum_out=se[:, 0:1])
                    rc = smp.tile([128, 1], F32, tag="rc")
                    nc.vector.reciprocal(rc[:, :], se[:, :])
                    rca = smp.tile([128, 1], F32, tag="rca")
                    nc.vector.tensor_mul(rca[:, :], rc[:, :], asb[:, g:g + 1])
                    wg = smp.tile([128, M], BF, tag="wg")
                    nc.vector.tensor_mul(wg[:, :], eqf[:, :],
                                         _bcast_m(rca[:, 0:1], M))
                    if last:
                        qg = smp.tile([128, M], F32, tag="qg")
                        nc.vector.tensor_mul(qg[:, :], eqf[:, :],
                                             _bcast_m(rc[:, 0:1], M))
                        nc.sync.dma_start(out=q_o[g], in_=qg[:, :])
                    # weighted votes rw = V * w (w broadcast over d, step 0)
                    rw = sp.tile([128, MD], BF, tag="scr")
                    rw_eng = nc.gpsimd if g in gps_rw_tiles else nc.vector
                    rw_eng.tensor_tensor(
                        rw.rearrange("p (m d) -> p m d", d=DOUT),
                        Vg.rearrange("p (m d) -> p m d", d=DOUT),
                        _bcast_md(wg[:, :], M, DOUT),
                        mybir.AluOpType.mult)
                    for c in range(8):
                        nc.tensor.matmul(
                            ncv_ps[:, c * 512:(c + 1) * 512],
                            lhsT=s4x[:, :],
                            rhs=rw[:, c * 512:(c + 1) * 512],
                            start=(g == 0), stop=(g == NT - 1),
                        )
                    if last:
                        # route_class_emb shard (bf16; upcast on host)
                        nc.sync.dma_start(out=rce_o[g], in_=rw[:, :])
                if not last:
                    ncvb = broadcast_ncv(ncv_ps, SCALE)
                else:
                    ncv_f = sp.tile([128, MD], BF, tag="scr", name="ncv_f")
                    nc.scalar.copy(ncv_f[0:BSH, :], ncv_ps[0:BSH, :])
                    nc.sync.dma_start(out=ncv_o[:, :], in_=ncv_f[0:BSH, :])

    nc.compile()
    return nc


_NC_CACHE: dict = {}


def _get_nc(num_iter: int) -> bass.Bass:
    if num_iter not in _NC_CACHE:
        _NC_CACHE[num_iter] = build_nc(num_iter)
    return _NC_CACHE[num_iter]


def _host_prep(x, current_act, W):
    x = np.asarray(x, dtype=np.float32)
    act = np.asarray(current_act, dtype=np.float32)
    W = np.asarray(W, dtype=np.float32)

    Wt = np.ascontiguousarray(W.reshape(NIN, DIN, MD).astype(BF16))

    p_ = np.arange(128)
    sel4x4 = (p_[:, None] % 32 == p_[None, :] % 32).astype(BF16)

    in_maps = []
    for core in range(NCORES):
        xs = x[core * BSH:(core + 1) * BSH]          # [32, 64, 128]
        acts = act[core * BSH:(core + 1) * BSH]      # [32, 64]
        # xT[a, (n, b)] = x[b, n, a]
        xT = np.ascontiguousarray(
            xs.transpose(2, 1, 0).reshape(DIN, NIN * BSH).astype(BF16))
        # act_t[p, g] = act[b, n] with b = p%32, n = 4g + p//32
        p = np.arange(128)
        g = np.arange(NT)
        act_t = np.ascontiguousarray(
            acts[p[:, None] % 32, 4 * g[None, :] + p[:, None] // 32]
            .astype(np.float32))
        in_maps.append({
            "xT": xT, "act_t": act_t, "Wt": Wt, "sel4x4": sel4x4,
        })
    return in_maps


def _host_post(results):
    ncv = np.empty((B, M, DOUT), dtype=np.float32)
    q = np.empty((B, NIN, M), dtype=np.float32)
    rce = np.empty((B, NIN, M, DOUT), dtype=np.float32)
    for core, res in enumerate(results):
        sl = slice(core * BSH, (core + 1) * BSH)
        ncv[sl] = res["ncv_o"].astype(np.float32).reshape(BSH, M, DOUT)
        # q_o[g, p, m] -> q[b, 4g + p//32, m], b = p%32
        q_t = res["q_o"].reshape(NT, 4, 32, M)          # [g, pn, b, m]
        q[sl] = q_t.transpose(2, 0, 1, 3).reshape(BSH, NIN, M)
        rce_t = res["rce_o"].astype(np.float32).reshape(NT, 4, 32, M, DOUT)
        rce[sl] = rce_t.transpose(2, 0, 1, 3, 4).reshape(BSH, NIN, M, DOUT)
    return ncv, q, rce


def kernel(x, current_act, W, num_iter=3, _trace=False, _tmpdir=None):
    num_iter = max(1, int(num_iter))
    nc = _get_nc(num_iter)
    in_maps = _host_prep(x, current_act, W)
    res = run_bass_kernel_spmd(
        nc, in_maps, list(range(NCORES)),
        trace=_trace, tmpdir=_tmpdir)
    out = _host_post(res.results)
    if _trace:
        return out, res
    return out


# revision 45
# speedup vs baseline: 1.1420x; 1.1420x over previous
"""Trainium2 Bass kernel for CapsuleFC EM-routing forward pass.

Shapes: x[256,64,128], current_act[256,64], W[64,128,32,128], num_iter=3.
Outputs: ncv[256,32,128], q[256,64,32], route_class_emb[256,64,32,128].

Strategy: data-parallel over batch across 8 cores (32 b per core), W replicated.
Per core, votes = einsum('bna,namd->bnmd') are computed once by streaming W
(bf16) through the PE and kept entirely in SBUF (bf16, 16.75 MB).  The three
EM-routing iterations then run out of SBUF: logits on DVE (mul + segmented
reduce), softmax on ACT (Exp with fused sum), weighted votes via per-m
tensor_scalar, and the sum over n on the PE (matmul against a 0/1 selector).
route_class_emb is the final weighted votes, cast-DMA'd out as fp32.

Partition layout for votes: partition p = (n mod 4)*32 + b, SBUF tile
g = n // 4 (16 tiles), free dim = (m, d) = 4096.
"""

import math
import numpy as np
import ml_dtypes
from contextlib import ExitStack

import concourse.bass as bass
import concourse.bacc as bacc
import concourse.mybir as mybir
from concourse.tile import TileContext
from concourse.bass_utils import run_bass_kernel_spmd

BF16 = ml_dtypes.bfloat16

B, NIN, DIN, M, DOUT = 256, 64, 128, 32, 128
NCORES = 8
BSH = B // NCORES          # 32 batch elements per core
MD = M * DOUT              # 4096
NT = NIN // 4              # 16 votes tiles (4 n's per tile)
SCALE = 1.0 / math.sqrt(DOUT)

F32 = mybir.dt.float32
BF = mybir.dt.bfloat16

# Tuning knobs.
# Of the 16 (b,n)-tiles per iteration: how many logits-reduces run on the
# Scalar engine (per-m Copy+accum) and how many logits-multiplies run on
# GpSimd, to offload the bottleneck Vector engine.
ACT_REDUCE_TILES = 0   # HW: ACT per-m accum reduce costs ~1.3us/m -> never
GPSIMD_MUL_TILES = 9   # logits multiplies offloaded to GpSimd
GPSIMD_RW_TILES = 0    # rw on GpSimd stalls the rw->matmul chain: keep 0


def _bcast_m(ap, m):
    """View a [128, 1] AP as [128, m] broadcast along free (step 0)."""
    return bass.AP(ap.tensor, ap.offset, [ap.ap[0], [0, m]])


def _bcast_md(ap, m, d):
    """View a [128, m] AP as [128, m, d] with d broadcast (step 0)."""
    return bass.AP(ap.tensor, ap.offset, [ap.ap[0], [ap.ap[-1][0], m], [0, d]])


def build_nc(num_iter: int = 3) -> bass.Bass:
    nc = bacc.Bacc()

    xT = nc.dram_tensor("xT", [DIN, NIN * BSH], BF, kind="ExternalInput")
    actt = nc.dram_tensor("act_t", [128, NT], F32, kind="ExternalInput")
    Wt = nc.dram_tensor("Wt", [NIN, DIN, MD], BF, kind="ExternalInput")
    # sel4x4[p, j] = (p%32 == j%32): matmuls with this as lhsT produce the
    # over-n reduction replicated to all four 32-partition groups at once
    sel4x4 = nc.dram_tensor("sel4x4", [128, 128], BF, kind="ExternalInput")

    ncv_o = nc.dram_tensor("ncv_o", [BSH, MD], BF, kind="ExternalOutput")
    q_o = nc.dram_tensor("q_o", [NT, 128, M], F32, kind="ExternalOutput")
    rce_o = nc.dram_tensor("rce_o", [NT, 128, MD], BF, kind="ExternalOutput")

    W_anm = Wt.rearrange("n a k -> a n k")  # [128, 64, 4096] strided DRAM view

    with TileContext(nc) as tc, ExitStack() as ctx:
        pp = ctx.enter_context(tc.tile_pool(name="persist", bufs=1))
        V = pp.tile([128, NT * MD], BF)            # votes, 128 KB/partition
        xsb = pp.tile([128, NIN * BSH], BF)        # x^T, free = (n, b)
        asb = pp.tile([128, NT], F32)              # act per (partition, tile)
        s4x = pp.tile([128, 128], BF)              # replicating selector
        lg = pp.tile([128, NT * M], F32)           # logits, free = (g, m)

        nc.sync.dma_start(out=xsb[:, :], in_=xT[:, :])
        nc.sync.dma_start(out=asb[:, :], in_=actt[:, :])
        nc.sync.dma_start(out=s4x[:, :], in_=sel4x4[:, :])

        # ---------------- Phase 1: votes = x @ W, streamed over n ----------
        with tc.tile_pool(name="wstream", bufs=3) as wp, \
             tc.tile_pool(name="p1ps", bufs=2, space="PSUM") as psp:
            for g in range(NT):
                wtiles = []
                for half in range(2):
                    n0 = 4 * g + 2 * half
                    wtile = wp.tile([128, 2 * MD], BF, tag="w", name=f"w_{g}_{half}")
                    for nl in range(2):
                        nc.sync.dma_start(
                            out=wtile[:, nl * MD:(nl + 1) * MD],
                            in_=W_anm[:, n0 + nl, :])
                    wtiles.append(wtile)
                # psA covers (m,d) chunks 0-3, psB chunks 4-7; each is written
                # by all four n's (col groups) of this quad.
                psA = psp.tile([128, 4 * 512], F32, tag="ps", name=f"psA_{g}")
                psB = psp.tile([128, 4 * 512], F32, tag="ps", name=f"psB_{g}")
                for half in range(2):
                    wtile = wtiles[half]
                    for nl in range(2):
                        n = 4 * g + 2 * half + nl
                        j = 2 * half + nl  # col group -> partitions 32j..32j+31
                        lhsT = xsb[:, n * BSH:(n + 1) * BSH]
                        for c in range(8):
                            ps = psA if c < 4 else psB
                            cl = c % 4
                            nc.tensor.matmul(
                                ps[32 * j:32 * j + 32, cl * 512:(cl + 1) * 512],
                                lhsT=lhsT,
                                rhs=wtile[:, nl * MD + c * 512: nl * MD + (c + 1) * 512],
                                start=True, stop=True,
                                tile_position=(0, 32 * j),
                            )
                # both PSUM->SBUF copies on ScalarE: phase 1 is DMA-bound and
                # this keeps the Vector engine free for the iteration phase
                nc.scalar.copy(
                    V[:, g * MD: g * MD + 2048], psA[:, :])
                nc.scalar.copy(
                    V[:, g * MD + 2048: g * MD + 4096], psB[:, :])

        with tc.tile_pool(name="it", bufs=1) as ip, \
             tc.tile_pool(name="scr", bufs=3) as sp, \
             tc.tile_pool(name="sm", bufs=4) as smp, \
             tc.tile_pool(name="itps", bufs=1, space="PSUM") as ipp:

            # ---------------- ncv_0 = (sum_n V) / M -------------------------
            # matmul against the replicating selector: output [128, MD] holds
            # the per-b sum over n replicated across all 4 partition groups
            ncv_ps = ipp.tile([128, MD], F32, tag="ncvps")
            for g in range(NT):
                for c in range(8):
                    nc.tensor.matmul(
                        ncv_ps[:, c * 512:(c + 1) * 512],
                        lhsT=s4x[:, :],
                        rhs=V[:, g * MD + c * 512: g * MD + (c + 1) * 512],
                        start=(g == 0), stop=(g == NT - 1),
                    )

            def broadcast_ncv(ncv_ps, scale):
                # single PSUM->SBUF copy; `scale` folds the logits SCALE (and
                # the 1/M of the init step) so logits need no extra scaling
                ncvb = ip.tile([128, MD], BF, tag="ncvb", bufs=2)
                nc.scalar.mul(ncvb[:, :], ncv_ps[:, :], scale)
                return ncvb

            ncvb = broadcast_ncv(ncv_ps, SCALE / M)

            # ---------------- routing iterations ---------------------------
            for it in range(num_iter):
                last = (it == num_iter - 1)
                ncv_ps = ipp.tile([128, MD], F32, tag="ncvps")
                # spread offloaded tiles through the loop so the pipeline
                # keeps flowing (gpsimd muls on odd tiles, gpsimd rw on
                # even tiles, ACT reduces on multiples of 4)
                gps_tiles = set(list(range(1, NT, 2))[:GPSIMD_MUL_TILES])
                gps_rw_tiles = set(list(range(0, NT, 2))[:GPSIMD_RW_TILES])
                act_tiles = set(list(range(0, NT, 4))[:ACT_REDUCE_TILES])
                for g in range(NT):
                    Vg = V[:, g * MD:(g + 1) * MD]
                    lgg = lg[:, g * M:(g + 1) * M]
                    # logits: mul + segmented reduce over d.  Free layout of
                    # votes is (d, m), so the reduce uses a strided [p, m, d]
                    # view and the w-broadcast below has innermost step 1.
                    tmp = sp.tile([128, MD], BF, tag="scr")
                    mul_eng = nc.gpsimd if g in gps_tiles else nc.vector
                    mul_eng.tensor_mul(tmp[:, :], Vg, ncvb[:, :])
                    tmp3 = tmp.rearrange("p (m d) -> p m d", d=DOUT)
                    if g in act_tiles:
                        # per-m Copy with fused accumulator on ScalarE
                        for m in range(M):
                            nc.scalar.activation(
                                tmp3[:, m, :], tmp3[:, m, :],
                                mybir.ActivationFunctionType.Copy,
                                accum_out=lgg[:, m:m + 1])
                    else:
                        nc.vector.reduce_sum(lgg, tmp3,
                                             axis=mybir.AxisListType.X)
                    # softmax over m (free dim); logits are pre-scaled by
                    # SCALE (folded into ncvb), max comes out negated.
                    nmx = smp.tile([128, 1], F32, tag="nmx")
                    nc.vector.tensor_reduce(
                        nmx[:, :], lgg, axis=mybir.AxisListType.X,
                        op=mybir.AluOpType.max, negate=True)
                    eqf = smp.tile([128, M], F32, tag="eqf")
                    se = smp.tile([128, 1], F32, tag="se")
                    nc.scalar.activation(
                        eqf[:, :], lgg, mybir.ActivationFunctionType.Exp,
                        bias=nmx[:, 0:1], scale=1.0, accum_out=se[:, 0:1])
                    rc = smp.tile([128, 1], F32, tag="rc")
                    nc.vector.reciprocal(rc[:, :], se[:, :])
                    rca = smp.tile([128, 1], F32, tag="rca")
                    nc.vector.tensor_mul(rca[:, :], rc[:, :], asb[:, g:g + 1])
                    wg = smp.tile([128, M], BF, tag="wg")
                    nc.vector.tensor_mul(wg[:, :], eqf[:, :],
                                         _bcast_m(rca[:, 0:1], M))
                    if last:
                        qg = smp.tile([128, M], F32, tag="qg")
                        nc.vector.tensor_mul(qg[:, :], eqf[:, :],
                                             _bcast_m(rc[:, 0:1], M))
                        nc.sync.dma_start(out=q_o[g], in_=qg[:, :])
                    # weighted votes rw = V * w (w broadcast over d, step 0)
                    rw = sp.tile([128, MD], BF, tag="scr")
                    rw_eng = nc.gpsimd if g in gps_rw_tiles else nc.vector
                    rw_eng.tensor_tensor(
                        rw.rearrange("p (m d) -> p m d", d=DOUT),
                        Vg.rearrange("p (m d) -> p m d", d=DOUT),
                        _bcast_md(wg[:, :], M, DOUT),
                        mybir.AluOpType.mult)
                    for c in range(8):
                        nc.tensor.matmul(
                            ncv_ps[:, c * 512:(c + 1) * 512],
                            lhsT=s4x[:, :],
                            rhs=rw[:, c * 512:(c + 1) * 512],
                            start=(g == 0), stop=(g == NT - 1),
                        )
                    if last:
                        # route_class_emb shard (bf16; upcast on host)
                        nc.sync.dma_start(out=rce_o[g], in_=rw[:, :])
                if not last:
                    ncvb = broadcast_ncv(ncv_ps, SCALE)
                else:
                    ncv_f = sp.tile([128, MD], BF, tag="scr", name="ncv_f")
                    nc.scalar.copy(ncv_f[0:BSH, :], ncv_ps[0:BSH, :])
                    nc.sync.dma_start(out=ncv_o[:, :], in_=ncv_f[0:BSH, :])

    nc.compile()
    return nc


_NC_CACHE: dict = {}


def _get_nc(num_iter: int) -> bass.Bass:
    if num_iter not in _NC_CACHE:
        _NC_CACHE[num_iter] = build_nc(num_iter)
    return _NC_CACHE[num_iter]


def _host_prep(x, current_act, W):
    x = np.asarray(x, dtype=np.float32)
    act = np.asarray(current_act, dtype=np.float32)
    W = np.asarray(W, dtype=np.float32)

    Wt = np.ascontiguousarray(W.reshape(NIN, DIN, MD).astype(BF16))

    p_ = np.arange(128)
    sel4x4 = (p_[:, None] % 32 == p_[None, :] % 32).astype(BF16)

    in_maps = []
    for core in range(NCORES):
        xs = x[core * BSH:(core + 1) * BSH]          # [32, 64, 128]
        acts = act[core * BSH:(core + 1) * BSH]      # [32, 64]
        # xT[a, (n, b)] = x[b, n, a]
        xT = np.ascontiguousarray(
            xs.transpose(2, 1, 0).reshape(DIN, NIN * BSH).astype(BF16))
        # act_t[p, g] = act[b, n] with b = p%32, n = 4g + p//32
        p = np.arange(128)
        g = np.arange(NT)
        act_t = np.ascontiguousarray(
            acts[p[:, None] % 32, 4 * g[None, :] + p[:, None] // 32]
            .astype(np.float32))
        in_maps.append({
            "xT": xT, "act_t": act_t, "Wt": Wt, "sel4x4": sel4x4,
        })
    return in_maps


def _host_post(results):
    ncv = np.empty((B, M, DOUT), dtype=np.float32)
    q = np.empty((B, NIN, M), dtype=np.float32)
    rce = np.empty((B, NIN, M, DOUT), dtype=np.float32)
    for core, res in enumerate(results):
        sl = slice(core * BSH, (core + 1) * BSH)
        ncv[sl] = res["ncv_o"].astype(np.float32).reshape(BSH, M, DOUT)
        # q_o[g, p, m] -> q[b, 4g + p//32, m], b = p%32
        q_t = res["q_o"].reshape(NT, 4, 32, M)          # [g, pn, b, m]
        q[sl] = q_t.transpose(2, 0, 1, 3).reshape(BSH, NIN, M)
        rce_t = res["rce_o"].astype(np.float32).reshape(NT, 4, 32, M, DOUT)
        rce[sl] = rce_t.transpose(2, 0, 1, 3, 4).reshape(BSH, NIN, M, DOUT)
    return ncv, q, rce


def kernel(x, current_act, W, num_iter=3, _trace=False, _tmpdir=None):
    num_iter = max(1, int(num_iter))
    nc = _get_nc(num_iter)
    in_maps = _host_prep(x, current_act, W)
    res = run_bass_kernel_spmd(
        nc, in_maps, list(range(NCORES)),
        trace=_trace, tmpdir=_tmpdir)
    out = _host_post(res.results)
    if _trace:
        return out, res
    return out


# revision 49
# speedup vs baseline: 1.4563x; 1.2751x over previous
"""Trainium2 Bass kernel for CapsuleFC EM-routing forward pass.

Shapes: x[256,64,128], current_act[256,64], W[64,128,32,128], num_iter=3.
Outputs: ncv[256,32,128], q[256,64,32], route_class_emb[256,64,32,128].

Strategy: data-parallel over batch across 8 cores (32 b per core), W replicated.
Per core, votes = einsum('bna,namd->bnmd') are computed once by streaming W
(bf16) through the PE and kept entirely in SBUF (bf16, 16.75 MB).  The three
EM-routing iterations then run out of SBUF: logits on DVE (mul + segmented
reduce), softmax on ACT (Exp with fused sum), weighted votes via per-m
tensor_scalar, and the sum over n on the PE (matmul against a 0/1 selector).
route_class_emb is the final weighted votes, cast-DMA'd out as fp32.

Partition layout for votes: partition p = (n mod 4)*32 + b, SBUF tile
g = n // 4 (16 tiles), free dim = (m, d) = 4096.
"""

import math
import numpy as np
import ml_dtypes
from contextlib import ExitStack

import concourse.bass as bass
import concourse.bacc as bacc
import concourse.mybir as mybir
from concourse.tile import TileContext
from concourse.bass_utils import run_bass_kernel_spmd

BF16 = ml_dtypes.bfloat16

B, NIN, DIN, M, DOUT = 256, 64, 128, 32, 128
NCORES = 8
BSH = B // NCORES          # 32 batch elements per core
MD = M * DOUT              # 4096
NT = NIN // 4              # 16 votes tiles (4 n's per tile)
SCALE = 1.0 / math.sqrt(DOUT)

F32 = mybir.dt.float32
BF = mybir.dt.bfloat16

# Tuning knobs.
# Of the 16 (b,n)-tiles per iteration: how many logits-reduces run on the
# Scalar engine (per-m Copy+accum) and how many logits-multiplies run on
# GpSimd, to offload the bottleneck Vector engine.
ACT_REDUCE_TILES = 0   # HW: ACT per-m accum reduce costs ~1.3us/m -> never
GPSIMD_MUL_TILES = 9   # logits multiplies offloaded to GpSimd
GPSIMD_RW_TILES = 0    # rw on GpSimd stalls the rw->matmul chain: keep 0


def _bcast_m(ap, m):
    """View a [128, 1] AP as [128, m] broadcast along free (step 0)."""
    return bass.AP(ap.tensor, ap.offset, [ap.ap[0], [0, m]])


def _w2_bcast(ap, m, d):
    """View a doubled [128, 2m] AP (adjacent value pairs) as
    [128, m, d/2, 2]: m outer (step 2), d/2 broadcast (step 0), innermost
    the packed pair (step 1) -- measured faster than a plain step-0 read."""
    return bass.AP(ap.tensor, ap.offset,
                   [ap.ap[0], [2, m], [0, d // 2], [1, 2]])


def build_nc(num_iter: int = 3) -> bass.Bass:
    nc = bacc.Bacc()

    xT = nc.dram_tensor("xT", [DIN, NIN * BSH], BF, kind="ExternalInput")
    actt = nc.dram_tensor("act_t", [128, NT], F32, kind="ExternalInput")
    Wt = nc.dram_tensor("Wt", [NIN, DIN, MD], BF, kind="ExternalInput")
    # sel4x4[p, j] = (p%32 == j%32): matmuls with this as lhsT produce the
    # over-n reduction replicated to all four 32-partition groups at once
    sel4x4 = nc.dram_tensor("sel4x4", [128, 128], BF, kind="ExternalInput")

    ncv_o = nc.dram_tensor("ncv_o", [BSH, MD], BF, kind="ExternalOutput")
    q_o = nc.dram_tensor("q_o", [128, NT * M], F32, kind="ExternalOutput")
    rce_o = nc.dram_tensor("rce_o", [NT, 128, MD], BF, kind="ExternalOutput")

    W_anm = Wt.rearrange("n a k -> a n k")  # [128, 64, 4096] strided DRAM view

    with TileContext(nc) as tc, ExitStack() as ctx:
        pp = ctx.enter_context(tc.tile_pool(name="persist", bufs=1))
        V = pp.tile([128, NT * MD], BF)            # votes, 128 KB/partition
        xsb = pp.tile([128, NIN * BSH], BF)        # x^T, free = (n, b)
        asb = pp.tile([128, NT], F32)              # act per (partition, tile)
        s4x = pp.tile([128, 128], BF)              # replicating selector
        lg = pp.tile([128, NT * M], F32)           # logits, free = (g, m)

        nc.sync.dma_start(out=xsb[:, :], in_=xT[:, :])
        nc.sync.dma_start(out=asb[:, :], in_=actt[:, :])
        nc.sync.dma_start(out=s4x[:, :], in_=sel4x4[:, :])

        # ---------------- Phase 1: votes = x @ W, streamed over n ----------
        with tc.tile_pool(name="wstream", bufs=3) as wp, \
             tc.tile_pool(name="p1ps", bufs=2, space="PSUM") as psp:
            for g in range(NT):
                wtiles = []
                for half in range(2):
                    n0 = 4 * g + 2 * half
                    wtile = wp.tile([128, 2 * MD], BF, tag="w", name=f"w_{g}_{half}")
                    for nl in range(2):
                        nc.sync.dma_start(
                            out=wtile[:, nl * MD:(nl + 1) * MD],
                            in_=W_anm[:, n0 + nl, :])
                    wtiles.append(wtile)
                # psA covers (m,d) chunks 0-3, psB chunks 4-7; each is written
                # by all four n's (col groups) of this quad.
                psA = psp.tile([128, 4 * 512], F32, tag="ps", name=f"psA_{g}")
                psB = psp.tile([128, 4 * 512], F32, tag="ps", name=f"psB_{g}")
                for half in range(2):
                    wtile = wtiles[half]
                    for nl in range(2):
                        n = 4 * g + 2 * half + nl
                        j = 2 * half + nl  # col group -> partitions 32j..32j+31
                        lhsT = xsb[:, n * BSH:(n + 1) * BSH]
                        for c in range(8):
                            ps = psA if c < 4 else psB
                            cl = c % 4
                            nc.tensor.matmul(
                                ps[32 * j:32 * j + 32, cl * 512:(cl + 1) * 512],
                                lhsT=lhsT,
                                rhs=wtile[:, nl * MD + c * 512: nl * MD + (c + 1) * 512],
                                start=True, stop=True,
                                tile_position=(0, 32 * j),
                            )
                # both PSUM->SBUF copies on ScalarE: phase 1 is DMA-bound and
                # this keeps the Vector engine free for the iteration phase
                nc.scalar.copy(
                    V[:, g * MD: g * MD + 2048], psA[:, :])
                nc.scalar.copy(
                    V[:, g * MD + 2048: g * MD + 4096], psB[:, :])

        with tc.tile_pool(name="it", bufs=1) as ip, \
             tc.tile_pool(name="scr", bufs=3) as sp, \
             tc.tile_pool(name="sm", bufs=4) as smp, \
             tc.tile_pool(name="itps", bufs=1, space="PSUM") as ipp:

            # ---------------- ncv_0 = (sum_n V) / M -------------------------
            # matmul against the replicating selector: output [128, MD] holds
            # the per-b sum over n replicated across all 4 partition groups
            ncv_ps = ipp.tile([128, MD], F32, tag="ncvps")
            for g in range(NT):
                for c in range(8):
                    nc.tensor.matmul(
                        ncv_ps[:, c * 512:(c + 1) * 512],
                        lhsT=s4x[:, :],
                        rhs=V[:, g * MD + c * 512: g * MD + (c + 1) * 512],
                        start=(g == 0), stop=(g == NT - 1),
                    )

            def broadcast_ncv(ncv_ps, scale):
                # single PSUM->SBUF copy; `scale` folds the logits SCALE (and
                # the 1/M of the init step) so logits need no extra scaling
                ncvb = ip.tile([128, MD], BF, tag="ncvb", bufs=2)
                nc.scalar.mul(ncvb[:, :], ncv_ps[:, :], scale)
                return ncvb

            ncvb = broadcast_ncv(ncv_ps, SCALE / M)

            # ---------------- routing iterations ---------------------------
            gps_tiles = set(list(range(1, NT, 2))[:GPSIMD_MUL_TILES])
            for it in range(num_iter):
                last = (it == num_iter - 1)
                ncv_ps = ipp.tile([128, MD], F32, tag="ncvps")
                # -- phase A: logits mul + segmented reduce, per tile --------
                for g in range(NT):
                    Vg = V[:, g * MD:(g + 1) * MD]
                    tmp = sp.tile([128, MD], BF, tag="scr")
                    mul_eng = nc.gpsimd if g in gps_tiles else nc.vector
                    mul_eng.tensor_mul(tmp[:, :], Vg, ncvb[:, :])
                    nc.vector.reduce_sum(
                        lg[:, g * M:(g + 1) * M],
                        tmp.rearrange("p (m d) -> p m d", d=DOUT),
                        axis=mybir.AxisListType.X)
                # -- phase B: softmax batched over all 16 tiles --------------
                # logits*SCALE are bounded (|.| < ~2), so exp needs no
                # max-subtraction; SCALE itself is folded into ncvb.
                eqf = smp.tile([128, NT * M], F32, tag="eqf", bufs=2)
                nc.scalar.activation(eqf[:, :], lg[:, :],
                                     mybir.ActivationFunctionType.Exp)
                se = smp.tile([128, NT], F32, tag="se")
                nc.vector.reduce_sum(
                    se[:, :], eqf.rearrange("p (g m) -> p g m", m=M),
                    axis=mybir.AxisListType.X)
                rc = smp.tile([128, NT], F32, tag="rc")
                nc.vector.reciprocal(rc[:, :], se[:, :])
                rca = smp.tile([128, NT], F32, tag="rca")
                nc.vector.tensor_mul(rca[:, :], rc[:, :], asb[:, :])
                # wg2[p, (g, m, 2)]: per-tile weights, each value doubled so
                # the rw read below streams packed pairs
                wg2 = smp.tile([128, NT * 2 * M], BF, tag="wg2", bufs=2)
                eq_dup = bass.AP(eqf.tensor, eqf.offset,
                                 [eqf.ap[0], [M, NT], [1, M], [0, 2]])
                rca_dup = bass.AP(rca.tensor, rca.offset,
                                  [rca.ap[0], [1, NT], [0, M], [0, 2]])
                nc.vector.tensor_tensor(
                    wg2.rearrange("p (g m two) -> p g m two", m=M, two=2),
                    eq_dup, rca_dup, mybir.AluOpType.mult)
                if last:
                    qa = smp.tile([128, NT * M], F32, tag="qa", bufs=1)
                    rc_b = bass.AP(rc.tensor, rc.offset,
                                   [rc.ap[0], [1, NT], [0, M]])
                    nc.vector.tensor_tensor(
                        qa.rearrange("p (g m) -> p g m", m=M),
                        eqf.rearrange("p (g m) -> p g m", m=M),
                        rc_b, mybir.AluOpType.mult)
                    nc.sync.dma_start(out=q_o[:, :], in_=qa[:, :])
                # -- phase C: weighted votes + ncv reduction, per tile -------
                for g in range(NT):
                    Vg = V[:, g * MD:(g + 1) * MD]
                    rw = sp.tile([128, MD], BF, tag="scr")
                    nc.vector.tensor_tensor(
                        rw.rearrange("p (m dh two) -> p m dh two",
                                     dh=DOUT // 2, two=2),
                        Vg.rearrange("p (m dh two) -> p m dh two",
                                     dh=DOUT // 2, two=2),
                        _w2_bcast(wg2[:, g * 2 * M:(g + 1) * 2 * M], M, DOUT),
                        mybir.AluOpType.mult)
                    for c in range(8):
                        nc.tensor.matmul(
                            ncv_ps[:, c * 512:(c + 1) * 512],
                            lhsT=s4x[:, :],
                            rhs=rw[:, c * 512:(c + 1) * 512],
                            start=(g == 0), stop=(g == NT - 1),
                        )
                    if last:
                        # route_class_emb shard (bf16; upcast on host)
                        nc.sync.dma_start(out=rce_o[g], in_=rw[:, :])
                if not last:
                    ncvb = broadcast_ncv(ncv_ps, SCALE)
                else:
                    ncv_f = sp.tile([128, MD], BF, tag="scr", name="ncv_f")
                    nc.scalar.copy(ncv_f[0:BSH, :], ncv_ps[0:BSH, :])
                    nc.sync.dma_start(out=ncv_o[:, :], in_=ncv_f[0:BSH, :])

    nc.compile()
    return nc


_NC_CACHE: dict = {}


def _get_nc(num_iter: int) -> bass.Bass:
    if num_iter not in _NC_CACHE:
        _NC_CACHE[num_iter] = build_nc(num_iter)
    return _NC_CACHE[num_iter]


def _host_prep(x, current_act, W):
    x = np.asarray(x, dtype=np.float32)
    act = np.asarray(current_act, dtype=np.float32)
    W = np.asarray(W, dtype=np.float32)

    Wt = np.ascontiguousarray(W.reshape(NIN, DIN, MD).astype(BF16))

    p_ = np.arange(128)
    sel4x4 = (p_[:, None] % 32 == p_[None, :] % 32).astype(BF16)

    in_maps = []
    for core in range(NCORES):
        xs = x[core * BSH:(core + 1) * BSH]          # [32, 64, 128]
        acts = act[core * BSH:(core + 1) * BSH]      # [32, 64]
        # xT[a, (n, b)] = x[b, n, a]
        xT = np.ascontiguousarray(
            xs.transpose(2, 1, 0).reshape(DIN, NIN * BSH).astype(BF16))
        # act_t[p, g] = act[b, n] with b = p%32, n = 4g + p//32
        p = np.arange(128)
        g = np.arange(NT)
        act_t = np.ascontiguousarray(
            acts[p[:, None] % 32, 4 * g[None, :] + p[:, None] // 32]
            .astype(np.float32))
        in_maps.append({
            "xT": xT, "act_t": act_t, "Wt": Wt, "sel4x4": sel4x4,
        })
    return in_maps


def _host_post(results):
    ncv = np.empty((B, M, DOUT), dtype=np.float32)
    q = np.empty((B, NIN, M), dtype=np.float32)
    rce = np.empty((B, NIN, M, DOUT), dtype=np.float32)
    for core, res in enumerate(results):
        sl = slice(core * BSH, (core + 1) * BSH)
        ncv[sl] = res["ncv_o"].astype(np.float32).reshape(BSH, M, DOUT)
        # q_o[p, (g, m)] -> q[b, 4g + p//32, m], b = p%32
        q_t = res["q_o"].reshape(4, 32, NT, M)          # [pn, b, g, m]
        q[sl] = q_t.transpose(1, 2, 0, 3).reshape(BSH, NIN, M)
        rce_t = res["rce_o"].astype(np.float32).reshape(NT, 4, 32, M, DOUT)
        rce[sl] = rce_t.transpose(2, 0, 1, 3, 4).reshape(BSH, NIN, M, DOUT)
    return ncv, q, rce


def kernel(x, current_act, W, num_iter=3, _trace=False, _tmpdir=None):
    num_iter = max(1, int(num_iter))
    nc = _get_nc(num_iter)
    in_maps = _host_prep(x, current_act, W)
    res = run_bass_kernel_spmd(
        nc, in_maps, list(range(NCORES)),
        trace=_trace, tmpdir=_tmpdir)
    out = _host_post(res.results)
    if _trace:
        return out, res
    return out


# revision 50
# speedup vs baseline: 1.4876x; 1.0215x over previous
"""Trainium2 Bass kernel for CapsuleFC EM-routing forward pass.

Shapes: x[256,64,128], current_act[256,64], W[64,128,32,128], num_iter=3.
Outputs: ncv[256,32,128], q[256,64,32], route_class_emb[256,64,32,128].

Strategy: data-parallel over batch across 8 cores (32 b per core), W replicated.
Per core, votes = einsum('bna,namd->bnmd') are computed once by streaming W
(bf16) through the PE and kept entirely in SBUF (bf16, 16.75 MB).  The three
EM-routing iterations then run out of SBUF: logits on DVE (mul + segmented
reduce), softmax on ACT (Exp with fused sum), weighted votes via per-m
tensor_scalar, and the sum over n on the PE (matmul against a 0/1 selector).
route_class_emb is the final weighted votes, cast-DMA'd out as fp32.

Partition layout for votes: partition p = (n mod 4)*32 + b, SBUF tile
g = n // 4 (16 tiles), free dim = (m, d) = 4096.
"""

import math
import numpy as np
import ml_dtypes
from contextlib import ExitStack

import concourse.bass as bass
import concourse.bacc as bacc
import concourse.mybir as mybir
from concourse.tile import TileContext
from concourse.bass_utils import run_bass_kernel_spmd

BF16 = ml_dtypes.bfloat16

B, NIN, DIN, M, DOUT = 256, 64, 128, 32, 128
NCORES = 8
BSH = B // NCORES          # 32 batch elements per core
MD = M * DOUT              # 4096
NT = NIN // 4              # 16 votes tiles (4 n's per tile)
SCALE = 1.0 / math.sqrt(DOUT)

F32 = mybir.dt.float32
BF = mybir.dt.bfloat16

# Tuning knobs.
# Of the 16 (b,n)-tiles per iteration: how many logits-reduces run on the
# Scalar engine (per-m Copy+accum) and how many logits-multiplies run on
# GpSimd, to offload the bottleneck Vector engine.
ACT_REDUCE_TILES = 0   # HW: ACT per-m accum reduce costs ~1.3us/m -> never
GPSIMD_MUL_TILES = 10  # logits multiplies offloaded to GpSimd
GPSIMD_RW_TILES = 0    # rw on GpSimd stalls the rw->matmul chain: keep 0


def _bcast_m(ap, m):
    """View a [128, 1] AP as [128, m] broadcast along free (step 0)."""
    return bass.AP(ap.tensor, ap.offset, [ap.ap[0], [0, m]])


def _w2_bcast(ap, m, d):
    """View a doubled [128, 2m] AP (adjacent value pairs) as
    [128, m, d/2, 2]: m outer (step 2), d/2 broadcast (step 0), innermost
    the packed pair (step 1) -- measured faster than a plain step-0 read."""
    return bass.AP(ap.tensor, ap.offset,
                   [ap.ap[0], [2, m], [0, d // 2], [1, 2]])


def build_nc(num_iter: int = 3) -> bass.Bass:
    nc = bacc.Bacc()

    xT = nc.dram_tensor("xT", [DIN, NIN * BSH], BF, kind="ExternalInput")
    actt = nc.dram_tensor("act_t", [128, NT], F32, kind="ExternalInput")
    Wt = nc.dram_tensor("Wt", [NIN, DIN, MD], BF, kind="ExternalInput")
    # sel4x4[p, j] = (p%32 == j%32): matmuls with this as lhsT produce the
    # over-n reduction replicated to all four 32-partition groups at once
    sel4x4 = nc.dram_tensor("sel4x4", [128, 128], BF, kind="ExternalInput")

    ncv_o = nc.dram_tensor("ncv_o", [BSH, MD], BF, kind="ExternalOutput")
    q_o = nc.dram_tensor("q_o", [128, NT * M], F32, kind="ExternalOutput")
    rce_o = nc.dram_tensor("rce_o", [NT, 128, MD], BF, kind="ExternalOutput")

    W_anm = Wt.rearrange("n a k -> a n k")  # [128, 64, 4096] strided DRAM view

    with TileContext(nc) as tc, ExitStack() as ctx:
        pp = ctx.enter_context(tc.tile_pool(name="persist", bufs=1))
        V = pp.tile([128, NT * MD], BF)            # votes, 128 KB/partition
        xsb = pp.tile([128, NIN * BSH], BF)        # x^T, free = (n, b)
        asb = pp.tile([128, NT], F32)              # act per (partition, tile)
        s4x = pp.tile([128, 128], BF)              # replicating selector
        lg = pp.tile([128, NT * M], F32)           # logits, free = (g, m)

        nc.sync.dma_start(out=xsb[:, :], in_=xT[:, :])
        nc.sync.dma_start(out=asb[:, :], in_=actt[:, :])
        nc.sync.dma_start(out=s4x[:, :], in_=sel4x4[:, :])

        # ---------------- Phase 1: votes = x @ W, streamed over n ----------
        with tc.tile_pool(name="wstream", bufs=3) as wp, \
             tc.tile_pool(name="p1ps", bufs=2, space="PSUM") as psp:
            for g in range(NT):
                wtiles = []
                for half in range(2):
                    n0 = 4 * g + 2 * half
                    wtile = wp.tile([128, 2 * MD], BF, tag="w", name=f"w_{g}_{half}")
                    for nl in range(2):
                        nc.sync.dma_start(
                            out=wtile[:, nl * MD:(nl + 1) * MD],
                            in_=W_anm[:, n0 + nl, :])
                    wtiles.append(wtile)
                # psA covers (m,d) chunks 0-3, psB chunks 4-7; each is written
                # by all four n's (col groups) of this quad.
                psA = psp.tile([128, 4 * 512], F32, tag="ps", name=f"psA_{g}")
                psB = psp.tile([128, 4 * 512], F32, tag="ps", name=f"psB_{g}")
                for half in range(2):
                    wtile = wtiles[half]
                    for nl in range(2):
                        n = 4 * g + 2 * half + nl
                        j = 2 * half + nl  # col group -> partitions 32j..32j+31
                        lhsT = xsb[:, n * BSH:(n + 1) * BSH]
                        for c in range(8):
                            ps = psA if c < 4 else psB
                            cl = c % 4
                            nc.tensor.matmul(
                                ps[32 * j:32 * j + 32, cl * 512:(cl + 1) * 512],
                                lhsT=lhsT,
                                rhs=wtile[:, nl * MD + c * 512: nl * MD + (c + 1) * 512],
                                start=True, stop=True,
                                tile_position=(0, 32 * j),
                            )
                # both PSUM->SBUF copies on ScalarE: phase 1 is DMA-bound and
                # this keeps the Vector engine free for the iteration phase
                nc.scalar.copy(
                    V[:, g * MD: g * MD + 2048], psA[:, :])
                nc.scalar.copy(
                    V[:, g * MD + 2048: g * MD + 4096], psB[:, :])

        with tc.tile_pool(name="it", bufs=1) as ip, \
             tc.tile_pool(name="scr", bufs=3) as sp, \
             tc.tile_pool(name="sm", bufs=4) as smp, \
             tc.tile_pool(name="itps", bufs=1, space="PSUM") as ipp:

            # ---------------- ncv_0 = (sum_n V) / M -------------------------
            # matmul against the replicating selector: output [128, MD] holds
            # the per-b sum over n replicated across all 4 partition groups
            ncv_ps = ipp.tile([128, MD], F32, tag="ncvps")
            for g in range(NT):
                for c in range(8):
                    nc.tensor.matmul(
                        ncv_ps[:, c * 512:(c + 1) * 512],
                        lhsT=s4x[:, :],
                        rhs=V[:, g * MD + c * 512: g * MD + (c + 1) * 512],
                        start=(g == 0), stop=(g == NT - 1),
                    )

            def broadcast_ncv(ncv_ps, scale):
                # single PSUM->SBUF copy; `scale` folds the logits SCALE (and
                # the 1/M of the init step) so logits need no extra scaling
                ncvb = ip.tile([128, MD], BF, tag="ncvb", bufs=2)
                nc.scalar.mul(ncvb[:, :], ncv_ps[:, :], scale)
                return ncvb

            ncvb = broadcast_ncv(ncv_ps, SCALE / M)

            # ---------------- routing iterations ---------------------------
            gps_tiles = set(list(range(1, NT, 2))[:GPSIMD_MUL_TILES])
            for it in range(num_iter):
                last = (it == num_iter - 1)
                ncv_ps = ipp.tile([128, MD], F32, tag="ncvps")
                # -- phase A: logits mul + segmented reduce, per tile --------
                for g in range(NT):
                    Vg = V[:, g * MD:(g + 1) * MD]
                    tmp = sp.tile([128, MD], BF, tag="scr")
                    mul_eng = nc.gpsimd if g in gps_tiles else nc.vector
                    mul_eng.tensor_mul(tmp[:, :], Vg, ncvb[:, :])
                    nc.vector.reduce_sum(
                        lg[:, g * M:(g + 1) * M],
                        tmp.rearrange("p (m d) -> p m d", d=DOUT),
                        axis=mybir.AxisListType.X)
                # -- phase B: softmax batched over all 16 tiles --------------
                # logits*SCALE are bounded (|.| < ~2), so exp needs no
                # max-subtraction; SCALE itself is folded into ncvb.
                eqf = smp.tile([128, NT * M], F32, tag="eqf", bufs=2)
                nc.scalar.activation(eqf[:, :], lg[:, :],
                                     mybir.ActivationFunctionType.Exp)
                se = smp.tile([128, NT], F32, tag="se")
                nc.vector.reduce_sum(
                    se[:, :], eqf.rearrange("p (g m) -> p g m", m=M),
                    axis=mybir.AxisListType.X)
                rc = smp.tile([128, NT], F32, tag="rc")
                nc.vector.reciprocal(rc[:, :], se[:, :])
                rca = smp.tile([128, NT], F32, tag="rca")
                nc.vector.tensor_mul(rca[:, :], rc[:, :], asb[:, :])
                # wg2[p, (g, m, 2)]: per-tile weights, each value doubled so
                # the rw read below streams packed pairs
                wg2 = smp.tile([128, NT * 2 * M], BF, tag="wg2", bufs=2)
                eq_dup = bass.AP(eqf.tensor, eqf.offset,
                                 [eqf.ap[0], [M, NT], [1, M], [0, 2]])
                rca_dup = bass.AP(rca.tensor, rca.offset,
                                  [rca.ap[0], [1, NT], [0, M], [0, 2]])
                nc.vector.tensor_tensor(
                    wg2.rearrange("p (g m two) -> p g m two", m=M, two=2),
                    eq_dup, rca_dup, mybir.AluOpType.mult)
                if last:
                    qa = smp.tile([128, NT * M], F32, tag="qa", bufs=1)
                    rc_b = bass.AP(rc.tensor, rc.offset,
                                   [rc.ap[0], [1, NT], [0, M]])
                    nc.vector.tensor_tensor(
                        qa.rearrange("p (g m) -> p g m", m=M),
                        eqf.rearrange("p (g m) -> p g m", m=M),
                        rc_b, mybir.AluOpType.mult)
                    nc.sync.dma_start(out=q_o[:, :], in_=qa[:, :])
                # -- phase C: weighted votes + ncv reduction, per tile -------
                for g in range(NT):
                    Vg = V[:, g * MD:(g + 1) * MD]
                    rw = sp.tile([128, MD], BF, tag="scr")
                    nc.vector.tensor_tensor(
                        rw.rearrange("p (m dh two) -> p m dh two",
                                     dh=DOUT // 2, two=2),
                        Vg.rearrange("p (m dh two) -> p m dh two",
                                     dh=DOUT // 2, two=2),
                        _w2_bcast(wg2[:, g * 2 * M:(g + 1) * 2 * M], M, DOUT),
                        mybir.AluOpType.mult)
                    for c in range(8):
                        nc.tensor.matmul(
                            ncv_ps[:, c * 512:(c + 1) * 512],
                            lhsT=s4x[:, :],
                            rhs=rw[:, c * 512:(c + 1) * 512],
                            start=(g == 0), stop=(g == NT - 1),
                        )
                    if last:
                        # route_class_emb shard (bf16; upcast on host)
                        nc.sync.dma_start(out=rce_o[g], in_=rw[:, :])
                if not last:
                    ncvb = broadcast_ncv(ncv_ps, SCALE)
                else:
                    ncv_f = sp.tile([128, MD], BF, tag="scr", name="ncv_f")
                    nc.scalar.copy(ncv_f[0:BSH, :], ncv_ps[0:BSH, :])
                    nc.sync.dma_start(out=ncv_o[:, :], in_=ncv_f[0:BSH, :])

    nc.compile()
    return nc


_NC_CACHE: dict = {}


def _get_nc(num_iter: int) -> bass.Bass:
    if num_iter not in _NC_CACHE:
        _NC_CACHE[num_iter] = build_nc(num_iter)
    return _NC_CACHE[num_iter]


def _host_prep(x, current_act, W):
    x = np.asarray(x, dtype=np.float32)
    act = np.asarray(current_act, dtype=np.float32)
    W = np.asarray(W, dtype=np.float32)

    Wt = np.ascontiguousarray(W.reshape(NIN, DIN, MD).astype(BF16))

    p_ = np.arange(128)
    sel4x4 = (p_[:, None] % 32 == p_[None, :] % 32).astype(BF16)

    in_maps = []
    for core in range(NCORES):
        xs = x[core * BSH:(core + 1) * BSH]          # [32, 64, 128]
        acts = act[core * BSH:(core + 1) * BSH]      # [32, 64]
        # xT[a, (n, b)] = x[b, n, a]
        xT = np.ascontiguousarray(
            xs.transpose(2, 1, 0).reshape(DIN, NIN * BSH).astype(BF16))
        # act_t[p, g] = act[b, n] with b = p%32, n = 4g + p//32
        p = np.arange(128)
        g = np.arange(NT)
        act_t = np.ascontiguousarray(
            acts[p[:, None] % 32, 4 * g[None, :] + p[:, None] // 32]
            .astype(np.float32))
        in_maps.append({
            "xT": xT, "act_t": act_t, "Wt": Wt, "sel4x4": sel4x4,
        })
    return in_maps


def _host_post(results):
    ncv = np.empty((B, M, DOUT), dtype=np.float32)
    q = np.empty((B, NIN, M), dtype=np.float32)
    rce = np.empty((B, NIN, M, DOUT), dtype=np.float32)
    for core, res in enumerate(results):
        sl = slice(core * BSH, (core + 1) * BSH)
        ncv[sl] = res["ncv_o"].astype(np.float32).reshape(BSH, M, DOUT)
        # q_o[p, (g, m)] -> q[b, 4g + p//32, m], b = p%32
        q_t = res["q_o"].reshape(4, 32, NT, M)          # [pn, b, g, m]
        q[sl] = q_t.transpose(1, 2, 0, 3).reshape(BSH, NIN, M)
        rce_t = res["rce_o"].astype(np.float32).reshape(NT, 4, 32, M, DOUT)
        rce[sl] = rce_t.transpose(2, 0, 1, 3, 4).reshape(BSH, NIN, M, DOUT)
    return ncv, q, rce


def kernel(x, current_act, W, num_iter=3, _trace=False, _tmpdir=None):
    num_iter = max(1, int(num_iter))
    nc = _get_nc(num_iter)
    in_maps = _host_prep(x, current_act, W)
    res = run_bass_kernel_spmd(
        nc, in_maps, list(range(NCORES)),
        trace=_trace, tmpdir=_tmpdir)
    out = _host_post(res.results)
    if _trace:
        return out, res
    return out


# revision 51
# speedup vs baseline: 1.4902x; 1.0017x over previous
"""Trainium2 Bass kernel for CapsuleFC EM-routing forward pass.

Shapes: x[256,64,128], current_act[256,64], W[64,128,32,128], num_iter=3.
Outputs: ncv[256,32,128], q[256,64,32], route_class_emb[256,64,32,128].

Strategy: data-parallel over batch across 8 cores (32 b per core), W replicated.
Per core, votes = einsum('bna,namd->bnmd') are computed once by streaming W
(bf16) through the PE and kept entirely in SBUF (bf16, 16.75 MB).  The three
EM-routing iterations then run out of SBUF: logits on DVE (mul + segmented
reduce), softmax on ACT (Exp with fused sum), weighted votes via per-m
tensor_scalar, and the sum over n on the PE (matmul against a 0/1 selector).
route_class_emb is the final weighted votes, cast-DMA'd out as fp32.

Partition layout for votes: partition p = (n mod 4)*32 + b, SBUF tile
g = n // 4 (16 tiles), free dim = (m, d) = 4096.
"""

import math
import numpy as np
import ml_dtypes
from contextlib import ExitStack

import concourse.bass as bass
import concourse.bacc as bacc
import concourse.mybir as mybir
from concourse.tile import TileContext
from concourse.bass_utils import run_bass_kernel_spmd

BF16 = ml_dtypes.bfloat16

B, NIN, DIN, M, DOUT = 256, 64, 128, 32, 128
NCORES = 8
BSH = B // NCORES          # 32 batch elements per core
MD = M * DOUT              # 4096
NT = NIN // 4              # 16 votes tiles (4 n's per tile)
SCALE = 1.0 / math.sqrt(DOUT)

F32 = mybir.dt.float32
BF = mybir.dt.bfloat16

# Tuning knobs.
# Of the 16 (b,n)-tiles per iteration: how many logits-reduces run on the
# Scalar engine (per-m Copy+accum) and how many logits-multiplies run on
# GpSimd, to offload the bottleneck Vector engine.
ACT_REDUCE_TILES = 0   # HW: ACT per-m accum reduce costs ~1.3us/m -> never
GPSIMD_MUL_TILES = 8   # logits multiplies offloaded to GpSimd
GPSIMD_RW_TILES = 0    # rw on GpSimd stalls the rw->matmul chain: keep 0


def _bcast_m(ap, m):
    """View a [128, 1] AP as [128, m] broadcast along free (step 0)."""
    return bass.AP(ap.tensor, ap.offset, [ap.ap[0], [0, m]])


def _w2_bcast(ap, m, d):
    """View a doubled [128, 2m] AP (adjacent value pairs) as
    [128, m, d/2, 2]: m outer (step 2), d/2 broadcast (step 0), innermost
    the packed pair (step 1) -- measured faster than a plain step-0 read."""
    return bass.AP(ap.tensor, ap.offset,
                   [ap.ap[0], [2, m], [0, d // 2], [1, 2]])


def build_nc(num_iter: int = 3) -> bass.Bass:
    nc = bacc.Bacc()

    xT = nc.dram_tensor("xT", [DIN, NIN * BSH], BF, kind="ExternalInput")
    actt = nc.dram_tensor("act_t", [128, NT], F32, kind="ExternalInput")
    Wt = nc.dram_tensor("Wt", [NIN, DIN, MD], BF, kind="ExternalInput")
    # sel4x4[p, j] = (p%32 == j%32): matmuls with this as lhsT produce the
    # over-n reduction replicated to all four 32-partition groups at once
    sel4x4 = nc.dram_tensor("sel4x4", [128, 128], BF, kind="ExternalInput")

    ncv_o = nc.dram_tensor("ncv_o", [BSH, MD], BF, kind="ExternalOutput")
    q_o = nc.dram_tensor("q_o", [128, NT * M], F32, kind="ExternalOutput")
    rce_o = nc.dram_tensor("rce_o", [NT, 128, MD], BF, kind="ExternalOutput")

    W_anm = Wt.rearrange("n a k -> a n k")  # [128, 64, 4096] strided DRAM view

    with TileContext(nc) as tc, ExitStack() as ctx:
        pp = ctx.enter_context(tc.tile_pool(name="persist", bufs=1))
        V = pp.tile([128, NT * MD], BF)            # votes, 128 KB/partition
        xsb = pp.tile([128, NIN * BSH], BF)        # x^T, free = (n, b)
        asb = pp.tile([128, NT], F32)              # act per (partition, tile)
        s4x = pp.tile([128, 128], BF)              # replicating selector
        lg = pp.tile([128, NT * M], F32)           # logits, free = (g, m)

        nc.sync.dma_start(out=xsb[:, :], in_=xT[:, :])
        nc.sync.dma_start(out=asb[:, :], in_=actt[:, :])
        nc.sync.dma_start(out=s4x[:, :], in_=sel4x4[:, :])

        # ---------------- Phase 1: votes = x @ W, streamed over n ----------
        with tc.tile_pool(name="wstream", bufs=4) as wp, \
             tc.tile_pool(name="p1ps", bufs=2, space="PSUM") as psp:
            for g in range(NT):
                wtiles = []
                for half in range(2):
                    n0 = 4 * g + 2 * half
                    wtile = wp.tile([128, 2 * MD], BF, tag="w", name=f"w_{g}_{half}")
                    for nl in range(2):
                        nc.sync.dma_start(
                            out=wtile[:, nl * MD:(nl + 1) * MD],
                            in_=W_anm[:, n0 + nl, :])
                    wtiles.append(wtile)
                # psA covers (m,d) chunks 0-3, psB chunks 4-7; each is written
                # by all four n's (col groups) of this quad.
                psA = psp.tile([128, 4 * 512], F32, tag="ps", name=f"psA_{g}")
                psB = psp.tile([128, 4 * 512], F32, tag="ps", name=f"psB_{g}")
                for half in range(2):
                    wtile = wtiles[half]
                    for nl in range(2):
                        n = 4 * g + 2 * half + nl
                        j = 2 * half + nl  # col group -> partitions 32j..32j+31
                        lhsT = xsb[:, n * BSH:(n + 1) * BSH]
                        for c in range(8):
                            ps = psA if c < 4 else psB
                            cl = c % 4
                            nc.tensor.matmul(
                                ps[32 * j:32 * j + 32, cl * 512:(cl + 1) * 512],
                                lhsT=lhsT,
                                rhs=wtile[:, nl * MD + c * 512: nl * MD + (c + 1) * 512],
                                start=True, stop=True,
                                tile_position=(0, 32 * j),
                            )
                # both PSUM->SBUF copies on ScalarE: phase 1 is DMA-bound and
                # this keeps the Vector engine free for the iteration phase
                nc.scalar.copy(
                    V[:, g * MD: g * MD + 2048], psA[:, :])
                nc.scalar.copy(
                    V[:, g * MD + 2048: g * MD + 4096], psB[:, :])

        with tc.tile_pool(name="it", bufs=1) as ip, \
             tc.tile_pool(name="scr", bufs=3) as sp, \
             tc.tile_pool(name="sm", bufs=4) as smp, \
             tc.tile_pool(name="itps", bufs=1, space="PSUM") as ipp:

            # ---------------- ncv_0 = (sum_n V) / M -------------------------
            # matmul against the replicating selector: output [128, MD] holds
            # the per-b sum over n replicated across all 4 partition groups
            ncv_ps = ipp.tile([128, MD], F32, tag="ncvps")
            for g in range(NT):
                for c in range(8):
                    nc.tensor.matmul(
                        ncv_ps[:, c * 512:(c + 1) * 512],
                        lhsT=s4x[:, :],
                        rhs=V[:, g * MD + c * 512: g * MD + (c + 1) * 512],
                        start=(g == 0), stop=(g == NT - 1),
                    )

            def broadcast_ncv(ncv_ps, scale):
                # single PSUM->SBUF copy; `scale` folds the logits SCALE (and
                # the 1/M of the init step) so logits need no extra scaling
                ncvb = ip.tile([128, MD], BF, tag="ncvb", bufs=2)
                nc.scalar.mul(ncvb[:, :], ncv_ps[:, :], scale)
                return ncvb

            ncvb = broadcast_ncv(ncv_ps, SCALE / M)

            # ---------------- routing iterations ---------------------------
            gps_tiles = set(list(range(1, NT, 2))[:GPSIMD_MUL_TILES])
            for it in range(num_iter):
                last = (it == num_iter - 1)
                ncv_ps = ipp.tile([128, MD], F32, tag="ncvps")
                # -- phase A: logits mul + segmented reduce, per tile --------
                for g in range(NT):
                    Vg = V[:, g * MD:(g + 1) * MD]
                    tmp = sp.tile([128, MD], BF, tag="scr")
                    mul_eng = nc.gpsimd if g in gps_tiles else nc.vector
                    mul_eng.tensor_mul(tmp[:, :], Vg, ncvb[:, :])
                    nc.vector.reduce_sum(
                        lg[:, g * M:(g + 1) * M],
                        tmp.rearrange("p (m d) -> p m d", d=DOUT),
                        axis=mybir.AxisListType.X)
                # -- phase B: softmax batched over all 16 tiles --------------
                # logits*SCALE are bounded (|.| < ~2), so exp needs no
                # max-subtraction; SCALE itself is folded into ncvb.
                eqf = smp.tile([128, NT * M], F32, tag="eqf", bufs=2)
                nc.scalar.activation(eqf[:, :], lg[:, :],
                                     mybir.ActivationFunctionType.Exp)
                se = smp.tile([128, NT], F32, tag="se")
                nc.vector.reduce_sum(
                    se[:, :], eqf.rearrange("p (g m) -> p g m", m=M),
                    axis=mybir.AxisListType.X)
                rc = smp.tile([128, NT], F32, tag="rc")
                nc.vector.reciprocal(rc[:, :], se[:, :])
                rca = smp.tile([128, NT], F32, tag="rca")
                nc.vector.tensor_mul(rca[:, :], rc[:, :], asb[:, :])
                # wg2[p, (g, m, 2)]: per-tile weights, each value doubled so
                # the rw read below streams packed pairs
                wg2 = smp.tile([128, NT * 2 * M], BF, tag="wg2", bufs=2)
                eq_dup = bass.AP(eqf.tensor, eqf.offset,
                                 [eqf.ap[0], [M, NT], [1, M], [0, 2]])
                rca_dup = bass.AP(rca.tensor, rca.offset,
                                  [rca.ap[0], [1, NT], [0, M], [0, 2]])
                nc.vector.tensor_tensor(
                    wg2.rearrange("p (g m two) -> p g m two", m=M, two=2),
                    eq_dup, rca_dup, mybir.AluOpType.mult)
                if last:
                    qa = smp.tile([128, NT * M], F32, tag="qa", bufs=1)
                    rc_b = bass.AP(rc.tensor, rc.offset,
                                   [rc.ap[0], [1, NT], [0, M]])
                    nc.vector.tensor_tensor(
                        qa.rearrange("p (g m) -> p g m", m=M),
                        eqf.rearrange("p (g m) -> p g m", m=M),
                        rc_b, mybir.AluOpType.mult)
                    nc.sync.dma_start(out=q_o[:, :], in_=qa[:, :])
                # -- phase C: weighted votes + ncv reduction, per tile -------
                for g in range(NT):
                    Vg = V[:, g * MD:(g + 1) * MD]
                    rw = sp.tile([128, MD], BF, tag="scr")
                    nc.vector.tensor_tensor(
                        rw.rearrange("p (m dh two) -> p m dh two",
                                     dh=DOUT // 2, two=2),
                        Vg.rearrange("p (m dh two) -> p m dh two",
                                     dh=DOUT // 2, two=2),
                        _w2_bcast(wg2[:, g * 2 * M:(g + 1) * 2 * M], M, DOUT),
                        mybir.AluOpType.mult)
                    for c in range(8):
                        nc.tensor.matmul(
                            ncv_ps[:, c * 512:(c + 1) * 512],
                            lhsT=s4x[:, :],
                            rhs=rw[:, c * 512:(c + 1) * 512],
                            start=(g == 0), stop=(g == NT - 1),
                        )
                    if last:
                        # route_class_emb shard (bf16; upcast on host)
                        nc.sync.dma_start(out=rce_o[g], in_=rw[:, :])
                if not last:
                    ncvb = broadcast_ncv(ncv_ps, SCALE)
                else:
                    ncv_f = sp.tile([128, MD], BF, tag="scr", name="ncv_f")
                    nc.scalar.copy(ncv_f[0:BSH, :], ncv_ps[0:BSH, :])
                    nc.sync.dma_start(out=ncv_o[:, :], in_=ncv_f[0:BSH, :])

    nc.compile()
    return nc


_NC_CACHE: dict = {}


def _get_nc(num_iter: int) -> bass.Bass:
    if num_iter not in _NC_CACHE:
        _NC_CACHE[num_iter] = build_nc(num_iter)
    return _NC_CACHE[num_iter]


def _host_prep(x, current_act, W):
    x = np.asarray(x, dtype=np.float32)
    act = np.asarray(current_act, dtype=np.float32)
    W = np.asarray(W, dtype=np.float32)

    Wt = np.ascontiguousarray(W.reshape(NIN, DIN, MD).astype(BF16))

    p_ = np.arange(128)
    sel4x4 = (p_[:, None] % 32 == p_[None, :] % 32).astype(BF16)

    in_maps = []
    for core in range(NCORES):
        xs = x[core * BSH:(core + 1) * BSH]          # [32, 64, 128]
        acts = act[core * BSH:(core + 1) * BSH]      # [32, 64]
        # xT[a, (n, b)] = x[b, n, a]
        xT = np.ascontiguousarray(
            xs.transpose(2, 1, 0).reshape(DIN, NIN * BSH).astype(BF16))
        # act_t[p, g] = act[b, n] with b = p%32, n = 4g + p//32
        p = np.arange(128)
        g = np.arange(NT)
        act_t = np.ascontiguousarray(
            acts[p[:, None] % 32, 4 * g[None, :] + p[:, None] // 32]
            .astype(np.float32))
        in_maps.append({
            "xT": xT, "act_t": act_t, "Wt": Wt, "sel4x4": sel4x4,
        })
    return in_maps


def _host_post(results):
    ncv = np.empty((B, M, DOUT), dtype=np.float32)
    q = np.empty((B, NIN, M), dtype=np.float32)
    rce = np.empty((B, NIN, M, DOUT), dtype=np.float32)
    for core, res in enumerate(results):
        sl = slice(core * BSH, (core + 1) * BSH)
        ncv[sl] = res["ncv_o"].astype(np.float32).reshape(BSH, M, DOUT)
        # q_o[p, (g, m)] -> q[b, 4g + p//32, m], b = p%32
        q_t = res["q_o"].reshape(4, 32, NT, M)          # [pn, b, g, m]
        q[sl] = q_t.transpose(1, 2, 0, 3).reshape(BSH, NIN, M)
        rce_t = res["rce_o"].astype(np.float32).reshape(NT, 4, 32, M, DOUT)
        rce[sl] = rce_t.transpose(2, 0, 1, 3, 4).reshape(BSH, NIN, M, DOUT)
    return ncv, q, rce


def kernel(x, current_act, W, num_iter=3, _trace=False, _tmpdir=None):
    num_iter = max(1, int(num_iter))
    nc = _get_nc(num_iter)
    in_maps = _host_prep(x, current_act, W)
    res = run_bass_kernel_spmd(
        nc, in_maps, list(range(NCORES)),
        trace=_trace, tmpdir=_tmpdir)
    out = _host_post(res.results)
    if _trace:
        return out, res
    return out
